# revision 46
# baseline (speedup 1.0000x reference)
"""GAT (3-layer, PPI-style) Bass/Tile kernel for 8 Trainium2 NeuronCores.

Strategy (graph/data parallel, per the dst-ownership sharding):
  - Nodes are sharded contiguously: core c owns nodes [c*NOWN, (c+1)*NOWN).
  - Edges live on the core owning dst; per core they are grouped by
    128-node dst groups and sorted so that edge-softmax segment reductions
    become small dense one-hot matmuls on the tensor engine.
  - Per layer: Phase A computes feat/el/er for owned nodes with one matmul
    against W_aug = [W | W@al_bd | W@ar_bd]; an AllGather publishes
    bf16 [feat, el] rows to every core; batched SWDGE dma_gather fetches the
    per-edge rows (feat+el by src from the gathered table, er by local dst);
    exp/leaky-relu run on ACT; out[n] = (sum_e ex_e * feat_src) / sum_e ex_e
    via one-hot matmuls (the softmax denominator rides along as extra matmul
    columns); ELU and a PE transpose produce the next layer's x^T.

All graph-dependent index structures are computed on the host inside
kernel() and shipped as tensor inputs, so one SPMD program serves all
8 cores. Gather calls carry exact valid-index counts (shared across
cores via max-padding); trailing slots use index -1, which SWDGE skips.
"""

import math

import numpy as np

try:
    from ml_dtypes import bfloat16 as np_bf16
except ImportError:  # pragma: no cover
    np_bf16 = None

P = 128
NCORES = 8


# ----------------------------------------------------------------------------
# Host-side preparation
# ----------------------------------------------------------------------------


def _wrap_idxs(idx, k):
    """int16 index array for dma_gather: wrapped in 16 partitions, replicated
    8x across the 128 partitions. idx: [k*128] -> [128, k*8]."""
    assert idx.shape[0] == k * P
    w = idx.astype(np.int16).reshape(k * 8, 16).T  # [16, k*8]
    return np.ascontiguousarray(np.tile(w, (8, 1)))  # [128, k*8]


def _prepare(inputs):
    h = np.asarray(inputs["h"], dtype=np.float32)
    src = np.asarray(inputs["src"]).astype(np.int64)
    dst = np.asarray(inputs["dst"]).astype(np.int64)

    N, NFEAT = h.shape
    E = src.shape[0]
    assert N % NCORES == 0
    NOWN = N // NCORES
    G = math.ceil(NOWN / P)
    HALF = (N + 1) // 2
    assert HALF <= 32767, "table half exceeds int16 gather index range"
    assert NOWN <= 32767

    Ws, als, ars = [], [], []
    for i in (1, 2, 3):
        Ws.append(np.asarray(inputs[f"W{i}"], dtype=np.float32))
        als.append(np.asarray(inputs[f"al{i}"], dtype=np.float32))
        ars.append(np.asarray(inputs[f"ar{i}"], dtype=np.float32))
    H = als[0].shape[0]
    FEAT = [W.shape[1] for W in Ws]  # H*D per layer
    D = [f // H for f in FEAT]
    NCLASS = D[-1]

    # W_aug = [W | W @ al_bd | W @ ar_bd] with al_bd[h*D+d, h] = al[h, d]
    Waug = []
    for W, al, ar, f, d in zip(Ws, als, ars, FEAT, D):
        al_bd = np.zeros((f, H), dtype=np.float32)
        ar_bd = np.zeros((f, H), dtype=np.float32)
        for hh in range(H):
            al_bd[hh * d : (hh + 1) * d, hh] = al[hh]
            ar_bd[hh * d : (hh + 1) * d, hh] = ar[hh]
        Waug.append(
            np.ascontiguousarray(
                np.concatenate([W, W @ al_bd, W @ ar_bd], axis=1)
            ).astype(np_bf16)
        )
    FO = [f + 2 * H for f in FEAT]
    # gather-table row widths in bf16 elements (row bytes multiple of 256)
    ROW = [math.ceil((f + H) * 2 / 256) * 128 for f in FEAT]

    # ---- edge partitioning --------------------------------------------------
    owner = dst // NOWN
    per_core = []
    cntA = np.zeros((NCORES, G), dtype=np.int64)
    cntB = np.zeros((NCORES, G), dtype=np.int64)
    for c in range(NCORES):
        sel = np.nonzero(owner == c)[0]
        e_src = src[sel]
        e_dst = dst[sel]
        dloc = e_dst - c * NOWN  # 0..NOWN-1
        grp = dloc // P  # dst group
        half = (e_src >= HALF).astype(np.int64)
        order = np.lexsort((e_src, half, grp))
        e_src, dloc, grp, half = e_src[order], dloc[order], grp[order], half[order]
        isA = half == 0
        cntA[c] = np.bincount(grp[isA], minlength=G)
        cntB[c] = np.bincount(grp[~isA], minlength=G)
        per_core.append((e_src, dloc))

    shA = cntA.max(axis=0)  # shared (cross-core) valid count per group
    shB = cntB.max(axis=0)
    kA = max(1, math.ceil(int(shA.max()) / P))
    kB = max(1, math.ceil(int(shB.max()) / P))
    K = kA + kB

    in_maps = []
    for c in range(NCORES):
        e_src, dloc = per_core[c]
        idxA = np.full((G, kA * P), -1, dtype=np.int64)
        idxB = np.full((G, kB * P), -1, dtype=np.int64)
        idxE = np.full((G, K * P), -1, dtype=np.int64)
        dstf = np.full((G, K * P), -1.0, dtype=np.float32)
        pos = 0
        for g in range(G):
            nA, nB = int(cntA[c, g]), int(cntB[c, g])
            sA = e_src[pos : pos + nA]
            dA = dloc[pos : pos + nA]
            sB = e_src[pos + nA : pos + nA + nB] - HALF
            dB = dloc[pos + nA : pos + nA + nB]
            pos += nA + nB
            # valid edges, then idx-0 fill up to the shared count (fetched but
            # masked via dstf=-1), then -1 (skipped by SWDGE)
            idxA[g, :nA] = sA
            idxA[g, nA : shA[g]] = 0
            idxB[g, :nB] = sB
            idxB[g, nB : shB[g]] = 0
            idxE[g, :nA] = dA
            idxE[g, nA : shA[g]] = 0
            idxE[g, kA * P : kA * P + nB] = dB
            idxE[g, kA * P + nB : kA * P + shB[g]] = 0
            dstf[g, :nA] = (dA - g * P).astype(np.float32)
            dstf[g, kA * P : kA * P + nB] = (dB - g * P).astype(np.float32)

        idxA_sb = np.concatenate([_wrap_idxs(idxA[g], kA) for g in range(G)], axis=1)
        idxB_sb = np.concatenate([_wrap_idxs(idxB[g], kB) for g in range(G)], axis=1)
        idxE_sb = np.concatenate([_wrap_idxs(idxE[g], K) for g in range(G)], axis=1)
        # dstf as SBUF layout [128, G*K]: [p, g*K+t] = dst_local of slot t*128+p
        dstf_sb = np.ascontiguousarray(dstf.reshape(G * K, P).T)

        hT = np.ascontiguousarray(h[c * NOWN : (c + 1) * NOWN, :].T).astype(np_bf16)

        m = {
            "hT": hT,
            "iota": np.broadcast_to(
                np.arange(P, dtype=np.float32)[None, :], (P, P)
            ).copy(),
            "iotar": np.broadcast_to(
                np.tile(np.arange(P, dtype=np.float32), K)[None, :],
                (P, K * P),
            ).copy(),
            "ident": np.eye(P, dtype=np.float32),
            "dstf": dstf_sb,
            "idxA": idxA_sb,
            "idxB": idxB_sb,
            "idxE": idxE_sb,
            "Wa1": Waug[0],
            "Wa2": Waug[1],
            "Wa3": Waug[2],
        }
        in_maps.append(m)

    cfg = dict(
        N=N,
        E=E,
        NFEAT=NFEAT,
        NOWN=NOWN,
        G=G,
        HALF=HALF,
        H=H,
        FEAT=FEAT,
        D=D,
        FO=FO,
        ROW=ROW,
        NCLASS=NCLASS,
        kA=kA,
        kB=kB,
        K=K,
        shA=tuple(int(x) for x in shA),
        shB=tuple(int(x) for x in shB),
    )
    return cfg, in_maps


# ----------------------------------------------------------------------------
# Bass program
# ----------------------------------------------------------------------------


def _build(cfg, repeat=1, ablate=()):
    import concourse.bacc as bacc
    import concourse.mybir as mybir
    import concourse.tile as tile

    ablate = frozenset(ablate)

    NOWN, G, HALF = cfg["NOWN"], cfg["G"], cfg["HALF"]
    N, NFEAT, H = cfg["N"], cfg["NFEAT"], cfg["H"]
    FEAT, FO, ROW, D = cfg["FEAT"], cfg["FO"], cfg["ROW"], cfg["D"]
    NCLASS = cfg["NCLASS"]
    kA, kB, K = cfg["kA"], cfg["kB"], cfg["K"]
    shA, shB = cfg["shA"], cfg["shB"]
    NEG = 0.2
    f32 = mybir.dt.float32
    bf16 = mybir.dt.bfloat16
    i16 = mybir.dt.int16
    AF = mybir.ActivationFunctionType
    OP = mybir.AluOpType

    F_IN = [NFEAT, FEAT[0], FEAT[1]]
    KT = [math.ceil(f / P) for f in F_IN]
    KTmax = max(KT)

    nc = bacc.Bacc(
        "TRN2",
        target_bir_lowering=False,
        debug=False,
        num_devices=NCORES,
        num_swdge_queues=4,
        dynamic_dma_scratch_size=32768,
    )

    # ---- I/O ----------------------------------------------------------------
    hT_d = nc.dram_tensor("hT", [NFEAT, NOWN], bf16, kind="ExternalInput")
    iota_d = nc.dram_tensor("iota", [P, P], f32, kind="ExternalInput")
    iotar_d = nc.dram_tensor("iotar", [P, K * P], f32, kind="ExternalInput")
    ident_d = nc.dram_tensor("ident", [P, P], f32, kind="ExternalInput")
    dstf_d = nc.dram_tensor("dstf", [P, G * K], f32, kind="ExternalInput")
    idxA_d = nc.dram_tensor("idxA", [P, G * kA * 8], i16, kind="ExternalInput")
    idxB_d = nc.dram_tensor("idxB", [P, G * kB * 8], i16, kind="ExternalInput")
    idxE_d = nc.dram_tensor("idxE", [P, G * K * 8], i16, kind="ExternalInput")
    W_d = [
        nc.dram_tensor(f"Wa{i + 1}", [F_IN[i], FO[i]], bf16, kind="ExternalInput")
        for i in range(3)
    ]
    out_d = nc.dram_tensor("out", [NOWN, NCLASS], f32, kind="ExternalOutput")

    # internal DRAM per layer
    ag_in = [
        nc.dram_tensor(f"ag_in{i}", [NOWN, ROW[i]], bf16, kind="Internal")
        for i in range(3)
    ]
    ag_out = [
        nc.dram_tensor(
            f"ag_out{i}", [NCORES * NOWN, ROW[i]], bf16, kind="Internal",
            addr_space="Shared",
        )
        for i in range(3)
    ]
    er_tab = [
        nc.dram_tensor(f"er_tab{i}", [G * P, 64], f32, kind="Internal")
        for i in range(3)
    ]

    rg = [list(range(NCORES))]

    with tile.TileContext(nc, num_cores=NCORES) as tc:
        with (
            tc.tile_pool(name="const", bufs=1) as cpool,
            tc.tile_pool(name="work", bufs=2) as wpool,
            tc.tile_pool(name="gath", bufs=6) as gpool,
            tc.tile_pool(name="psum", bufs=2, space="PSUM") as pspool,
        ):
            iota_t = cpool.tile([P, P], f32, name="iota_t")
            iotar_t = cpool.tile([P, K * P], f32, name="iotar_t")
            nc.sync.dma_start(iotar_t[:], iotar_d[:])
            ident_t = cpool.tile([P, P], f32, name="ident_t")
            dstf_t = cpool.tile([P, G * K], f32, name="dstf_t")
            idxA_t = cpool.tile([P, G * kA * 8], i16, name="idxA_t")
            idxB_t = cpool.tile([P, G * kB * 8], i16, name="idxB_t")
            idxE_t = cpool.tile([P, G * K * 8], i16, name="idxE_t")
            nc.sync.dma_start(iota_t[:], iota_d[:])
            nc.sync.dma_start(ident_t[:], ident_d[:])
            nc.sync.dma_start(dstf_t[:], dstf_d[:])
            nc.sync.dma_start(idxA_t[:], idxA_d[:])
            nc.sync.dma_start(idxB_t[:], idxB_d[:])
            nc.sync.dma_start(idxE_t[:], idxE_d[:])

            W_t = []
            for l in range(3):
                slices = []
                for k in range(KT[l]):
                    r0 = k * P
                    r1 = min(r0 + P, F_IN[l])
                    w = cpool.tile([P, FO[l]], bf16, name=f"W{l}_{k}")
                    nc.sync.dma_start(w[: r1 - r0, :], W_d[l][r0:r1, :])
                    slices.append(w)
                W_t.append(slices)

            # x^T tiles, [128, NOWN] per 128-row slice of the input features
            xT = [
                cpool.tile([P, NOWN], bf16, name=f"xT{k}") for k in range(KTmax)
            ]
            for k in range(KT[0]):
                r0, r1 = k * P, min((k + 1) * P, NFEAT)
                nc.sync.dma_start(xT[k][: r1 - r0, :], hT_d[r0:r1, :])

            er_big = cpool.tile([P, G * 64], f32, name="er_big")

            for _rep in range(repeat):
                for l in range(3):
                    FT, FOL, RW, DL = FEAT[l], FO[l], ROW[l], D[l]
                    last = l == 2

                    # ---------------- Phase A: feat/el/er for owned nodes ---
                    nc.vector.memset(er_big[:], 0.0)
                    for g in range(G):
                        nn = min(P, NOWN - g * P)
                        psA = pspool.tile([P, FOL], f32, name="psA", tag="psA")
                        for k in range(KT[l]):
                            kk = min(P, F_IN[l] - k * P)
                            lhs = xT[k][:kk, g * P : g * P + nn]
                            rhs = W_t[l][k][:kk, :]
                            nc.tensor.matmul(
                                psA[:nn, :],
                                lhsT=lhs,
                                rhs=rhs,
                                start=(k == 0),
                                stop=(k == KT[l] - 1),
                            )
                        stage = wpool.tile([P, RW], bf16, name="stage", tag="stage")
                        nc.vector.tensor_copy(
                            stage[:nn, 0 : FT + H], psA[:nn, 0 : FT + H]
                        )
                        nc.vector.tensor_copy(
                            er_big[:nn, g * 64 : g * 64 + H], psA[:nn, FT + H : FOL]
                        )
                        nc.sync.dma_start(
                            ag_in[l][g * P : g * P + nn, :], stage[:nn, :]
                        )
                    # er table: [128, G*64] -> [G*128, 64]
                    nc.sync.dma_start(
                        er_tab[l][:].rearrange("(g p) c -> p g c", p=P),
                        er_big[:].rearrange("p (g c) -> p g c", c=64),
                    )

                    # ---------------- AllGather ----------------------------
                    if "noag" not in ablate:
                        nc.gpsimd.collective_compute(
                            "AllGather",
                            mybir.AluOpType.bypass,
                            replica_groups=rg,
                            ins=[ag_in[l][:]],
                            outs=[ag_out[l][:]],
                        )

                    tabA = ag_out[l][0:HALF, :]
                    tabB = ag_out[l][HALF:N, :]

                    # ---------------- Edge phase (paired dst groups) --------
                    # 3 gather calls per pair of groups (feat-A, feat-B, er),
                    # multi-packet; group epilogues are software-pipelined one
                    # group behind the one-hot matmuls to keep engines busy.
                    FW = FT + H

                    # single_packet packs <=64 descriptors (7 tiles) per SDMA
                    # engine per call; chunk larger sections into 7-tile calls
                    GCH = 7

                    def emit_gather(dst3, tab, idx_col0, ktiles, row, valid, q):
                        idx_t = (idxA_t, idxB_t, idxE_t, idxE_t)[q]
                        t0 = 0
                        while t0 < ktiles:
                            tc_ = min(GCH, ktiles - t0)
                            cnt = min(max(valid - t0 * P, 0), tc_ * P)
                            if cnt == 0:
                                break
                            nc.gpsimd.dma_gather(
                                dst3[:, t0 : t0 + tc_, :],
                                tab,
                                idx_t[
                                    :,
                                    idx_col0 + t0 * 8 : idx_col0 + (t0 + tc_) * 8,
                                ],
                                tc_ * P,
                                cnt,
                                row,
                                elem_step=row,
                                queue_num=q,
                                single_packet=True,
                            )
                            t0 += tc_

                    def emit_epi(g, ps_c):
                        nn = min(P, NOWN - g * P)
                        s_r = wpool.tile([P, H], f32, name="s_r", tag="s_r")
                        nc.vector.tensor_scalar_max(
                            s_r[:], ps_c[:, FT:FW], 1e-30
                        )
                        nc.vector.reciprocal(s_r[:], s_r[:])
                        if last:
                            nc.vector.tensor_scalar_mul(s_r[:], s_r[:], 1.0 / H)
                        xg = wpool.tile([P, FT], f32, name="xg", tag="xg")
                        nc.vector.tensor_mul(
                            xg[:].rearrange("p (h d) -> p h d", h=H),
                            ps_c[:, 0:FT].rearrange("p (h d) -> p h d", h=H),
                            s_r[:].to_broadcast([P, H, DL]),
                        )
                        if not last:
                            # elu(x) = max(x, exp(min(x, 0)) - 1), transpose
                            mg = wpool.tile([P, FT], f32, name="mg", tag="mg")
                            nc.vector.tensor_scalar_min(mg[:], xg[:], 0.0)
                            nc.scalar.activation(mg[:], mg[:], AF.Exp)
                            nc.vector.scalar_tensor_tensor(
                                out=xg[:],
                                in0=mg[:],
                                scalar=-1.0,
                                in1=xg[:],
                                op0=OP.add,
                                op1=OP.max,
                            )
                            for kk in range(KT[l + 1]):
                                c0 = kk * P
                                c1 = min(c0 + P, FT)
                                w = c1 - c0
                                pt = pspool.tile([P, P], f32, name="pt", tag="pt")
                                nc.tensor.transpose(
                                    pt[:w, :], xg[:, c0:c1], ident_t[:]
                                )
                                nc.vector.tensor_copy(
                                    xT[kk][:w, g * P : g * P + nn], pt[:w, :nn]
                                )
                        else:
                            # mean over heads -> [nn, NCLASS] -> DRAM
                            o1 = wpool.tile([P, NCLASS], f32, name="o1", tag="o1")
                            o2 = wpool.tile([P, NCLASS], f32, name="o2", tag="o2")
                            nc.vector.tensor_add(
                                o1[:], xg[:, 0:NCLASS], xg[:, NCLASS : 2 * NCLASS]
                            )
                            nc.vector.tensor_add(
                                o2[:],
                                xg[:, 2 * NCLASS : 3 * NCLASS],
                                xg[:, 3 * NCLASS : 4 * NCLASS],
                            )
                            nc.vector.tensor_add(o1[:], o1[:], o2[:])
                            nc.sync.dma_start(
                                out_d[g * P : g * P + nn, :], o1[:nn, :]
                            )

                    GE = 0 if "noedge" in ablate else (8 if "g8" in ablate else G)
                    gathers_on = not ({"conly", "nofeat"} & ablate)
                    compute_on = "gonly" not in ablate
                    pend = None
                    for g in range(GE):
                        fb = gpool.tile([P, K * RW], bf16, name="fb", tag="fb")
                        eb = gpool.tile([P, K * 64], f32, name="eb", tag="eb")
                        if (g < 6 and _rep == 0) or not gathers_on:
                            # stale (index -1) slots must hold finite data:
                            # 0*NaN would poison the one-hot matmul PSUM
                            nc.vector.memset(fb[:], 0.0)
                            nc.vector.memset(eb[:], 0.0)
                        f3 = fb[:].rearrange("p (k r) -> p k r", r=RW)
                        e3 = eb[:].rearrange("p (k r) -> p k r", r=64)
                        if gathers_on:
                            if "noerg" not in ablate:
                                emit_gather(
                                    e3[:, 0:kA, :], er_tab[l][:], g * K * 8, kA,
                                    64, shA[g], 2,
                                )
                                emit_gather(
                                    e3[:, kA:K, :], er_tab[l][:],
                                    g * K * 8 + kA * 8, kB, 64, shB[g], 3,
                                )
                            if "nofeatg" not in ablate:
                                emit_gather(
                                    f3[:, 0:kA, :], tabA, g * kA * 8, kA, RW,
                                    shA[g], 0,
                                )
                                emit_gather(
                                    f3[:, kA:K, :], tabB, g * kB * 8, kB, RW,
                                    shB[g], 1,
                                )
                        if not compute_on:
                            continue

                        # gather-independent DVE work first: the one-hot build
                        # and the previous group's epilogue run while this
                        # group's feat rows are still in flight (DVE executes
                        # in emission order — a gather-dependent op at the head
                        # would stall everything behind it)
                        oh = wpool.tile([P, K * P], bf16, name="oh", tag="oh")
                        nc.vector.tensor_tensor(
                            out=oh[:].rearrange("p (k q) -> p k q", q=P),
                            in0=dstf_t[:, g * K : (g + 1) * K].rearrange(
                                "p (k o) -> p k o", o=1
                            ).to_broadcast([P, K, P]),
                            in1=iotar_t[:].rearrange("p (k q) -> p k q", q=P),
                            op=OP.is_equal,
                        )
                        if pend is not None:
                            emit_epi(*pend)
                            pend = None

                        # e = exp(leaky_relu(el + er)) for all K tiles at once
                        ee = wpool.tile([P, K * H], f32, name="ee", tag="ee")
                        nc.vector.tensor_add(
                            ee[:].rearrange("p (k h) -> p k h", h=H),
                            f3[:, :, FT:FW],
                            e3[:, :, 0:H],
                        )
                        # leaky_relu(x) = max(0.2*x, x)
                        nc.vector.scalar_tensor_tensor(
                            out=ee[:], in0=ee[:], scalar=NEG, in1=ee[:],
                            op0=OP.mult, op1=OP.max,
                        )
                        eeb = wpool.tile([P, K * H], bf16, name="eeb", tag="eeb")
                        nc.scalar.activation(eeb[:], ee[:], AF.Exp)

                        fs = wpool.tile([P, K * FW], bf16, name="fs", tag="fs")
                        eeb3 = eeb[:].rearrange("p (k h) -> p k h", h=H)
                        fs3 = fs[:].rearrange("p (k f) -> p k f", f=FW)
                        nc.vector.tensor_mul(
                            fs3[:, :, 0:FT].rearrange("p k (h d) -> p k h d", h=H),
                            f3[:, :, 0:FT].rearrange("p k (h d) -> p k h d", h=H),
                            eeb3.rearrange(
                                "p k (h o) -> p k h o", o=1
                            ).to_broadcast([P, K, H, DL]),
                        )
                        nc.vector.tensor_copy(fs3[:, :, FT:FW], eeb3)

                        ps_c = pspool.tile(
                            [P, FW], f32, name="ps_c", tag="ps_c", bufs=3
                        )
                        if "nomm" not in ablate:
                            for t in range(K):
                                nc.tensor.matmul(
                                    ps_c[:],
                                    lhsT=oh[:, t * P : (t + 1) * P],
                                    rhs=fs[:, t * FW : (t + 1) * FW],
                                    start=(t == 0),
                                    stop=(t == K - 1),
                                )
                        pend = (g, ps_c)
                    if pend is not None:
                        emit_epi(*pend)
                        pend = None

    nc.compile()
    return nc


# ----------------------------------------------------------------------------
# Driver
# ----------------------------------------------------------------------------

_CACHE = {}


def _get_nc(cfg, repeat=1, ablate=()):
    key = str(sorted(cfg.items())) + str(repeat) + str(sorted(ablate))
    if key not in _CACHE:
        _CACHE[key] = _build(cfg, repeat=repeat, ablate=ablate)
    return _CACHE[key]


def _run(inputs, use_sim=False, bench_iters=0, repeat=1, ablate=()):
    cfg, in_maps = _prepare(inputs)
    if "idx0" in ablate:
        # timing probe: keep descriptor counts/skips, but fetch row 0 always
        for m in in_maps:
            for k in ("idxA", "idxB", "idxE"):
                m[k] = np.where(m[k] < 0, m[k], 0).astype(np.int16)
    nc = _get_nc(cfg, repeat, tuple(a for a in ablate if a != "idx0"))

    if use_sim:
        from concourse.bass_interp import MultiCoreSim

        sim = MultiCoreSim(nc, num_cores=NCORES, require_finite=False)
        for c in range(NCORES):
            for k, v in in_maps[c].items():
                sim.cores[c].tensor(k)[:] = v
        sim.simulate(check_with_hw=False)
        outs = [np.array(sim.cores[c].tensor("out")) for c in range(NCORES)]
        res = None
    else:
        outs, res = _pjrt_run(nc, in_maps, bench_iters=bench_iters)

    out = np.concatenate(outs, axis=0).astype(np.float32)
    return out, res


def _pjrt_run(nc, in_maps, bench_iters=0):
    """Execute the SPMD program on the 8 axon-tunneled cores via PJRT.

    Mirrors concourse.bass2jax.run_bass_via_pjrt but keeps the compiled
    executable so warm re-runs can be timed (bench_iters > 0)."""
    import time as _time

    import jax
    import numpy as _np
    from jax.sharding import Mesh, PartitionSpec
    from jax.experimental.shard_map import shard_map

    import concourse.mybir as mybir
    from concourse.bass2jax import (
        _bass_exec_p,
        install_neuronx_cc_hook,
        partition_id_tensor,
    )

    install_neuronx_cc_hook()
    n_cores = len(in_maps)

    partition_name = nc.partition_id_tensor.name if nc.partition_id_tensor else None
    in_names, out_names, out_avals, zero_outs = [], [], [], []
    for alloc in nc.m.functions[0].allocations:
        if not isinstance(alloc, mybir.MemoryLocationSet):
            continue
        name = alloc.memorylocations[0].name
        if alloc.kind == "ExternalInput":
            if name != partition_name:
                in_names.append(name)
        elif alloc.kind == "ExternalOutput":
            shape = tuple(alloc.tensor_shape)
            dtype = mybir.dt.np(alloc.dtype)
            out_names.append(name)
            out_avals.append(jax.core.ShapedArray(shape, dtype))
            zero_outs.append(_np.zeros(shape, dtype))
    n_params = len(in_names)
    n_outs = len(out_avals)
    in_names_all = list(in_names) + list(out_names)
    if partition_name is not None:
        in_names_all.append(partition_name)
    donate = tuple(range(n_params, n_params + n_outs))

    def _body(*args):
        operands = list(args)
        if partition_name is not None:
            operands.append(partition_id_tensor())
        outs = _bass_exec_p.bind(
            *operands,
            out_avals=tuple(out_avals),
            in_names=tuple(in_names_all),
            out_names=tuple(out_names),
            lowering_input_output_aliases=(),
            sim_require_finite=True,
            sim_require_nnan=True,
            nc=nc,
        )
        return tuple(outs)

    devices = jax.devices()[:n_cores]
    mesh = Mesh(_np.asarray(devices), ("core",))
    in_specs = (PartitionSpec("core"),) * (n_params + n_outs)
    out_specs = (PartitionSpec("core"),) * n_outs
    sharded = jax.jit(
        shard_map(
            _body, mesh=mesh, in_specs=in_specs, out_specs=out_specs,
            check_rep=False,
        ),
        donate_argnums=donate,
        keep_unused=True,
    )
    concat_in = [
        _np.concatenate([_np.asarray(in_maps[c][nm]) for c in range(n_cores)], axis=0)
        for nm in in_names
    ]

    def _zeros_dev():
        return [
            jax.device_put(
                _np.zeros((n_cores * z.shape[0], *z.shape[1:]), z.dtype),
                jax.sharding.NamedSharding(mesh, PartitionSpec("core")),
            )
            for z in zero_outs
        ]

    dev_in = [
        jax.device_put(a, jax.sharding.NamedSharding(mesh, PartitionSpec("core")))
        for a in concat_in
    ]

    out_arrs = sharded(*dev_in, *_zeros_dev())
    jax.block_until_ready(out_arrs)

    times = []
    for _ in range(bench_iters):
        zs = _zeros_dev()
        jax.block_until_ready(zs)
        t0 = _time.perf_counter()
        o = sharded(*dev_in, *zs)
        jax.block_until_ready(o)
        times.append(_time.perf_counter() - t0)

    outs = [
        {
            nm: _np.asarray(out_arrs[i]).reshape(n_cores, *out_avals[i].shape)[c]
            for i, nm in enumerate(out_names)
        }
        for c in range(n_cores)
    ]
    res = {"times_s": times, "min_time_ns": int(min(times) * 1e9) if times else None}
    return [o["out"] for o in outs], res


def kernel(**inputs):
    out, _ = _run(inputs)
    return out
